# revision 7
# baseline (speedup 1.0000x reference)
"""Trainium2 Bass kernel for GQA attention (B=2, S=2048, D=2048, H=32, KVH=8).

Sharding: 8 cores = 2 batches x 4 head-groups. Each core handles one batch and
8 q-heads / 2 kv-heads: wq/wk/wv column-parallel, wo row-parallel; the partial
wo products are summed on the host.

Host-side prep (pure layout, no math): inputs are sharded, head-permuted and
pre-transposed so every matmul operand DMAs straight into its [K-on-partition]
layout; cos/sin of the rope angles are also computed host-side (the ScalarE Sin
LUT only covers [-pi, pi]).

Per-core kernel (all matmuls bf16/f32r):
  - q/k/v projections computed with s on partitions ([s,o] layout) from the
    pre-transposed xT/wqT/wkvT, RoPE applied with strided DVE ops writing bf16,
    then q/k transposed to [o,s] with the DMA xbar (off the PE).
  - scores are computed transposed: scT[k,q] = kT.T @ qT per head; exp on ACT;
    causal handled by skipping fully-masked k-tiles + one merged affine_select
    per diagonal tile (both head-halves in one gpsimd op).
  - PV: per (pair-group, pair) one [65,1024] PSUM accumulator; lhsT =
    [v_head | ones] (M=65) so the softmax denominator accumulates in PSUM row
    64 alongside the output.
  - normalization is DEFERRED and BATCHED per chunk: each pair's Z row is
    DMA-gathered into a [4,1024] tile; one Ln + one Exp (sharing the scores'
    ACT table set -> no table reloads) produce Zinv for all 4 pairs; the
    Zinv partition-broadcasts (K=1 f32r matmuls) + multiplies are emitted as
    pull-able quanta into the NEXT chunk.
  - final: res[s,d] = sum_p attnT_p.T @ woT_p, accumulated over 4 o-blocks.

Scheduling: the PE queue executes in order, so exp (ACT) latency inside the
attention j-loops is hidden by FINE-GRAINED interleaving: the projection of
chunk c+1, the final matmuls of chunk c-1 and the deferred normalization of
chunk c-1 are chopped into single-matmul "quanta" kept in a deque, and a
paced number of quanta is pulled between the scores and PV matmuls of every
j-step. Head order within a core is permuted to [0,4,1,5,2,6,3,7] so each
128-partition block pairs head h (kv0) with h+4 (kv1), letting the K=64 score
matmuls row-pack two heads concurrently on the PE array.

PSUM budget (8 banks): scores [128,1024] x2 bufs = 4, PV [65,1024] x1 = 2,
shared proj/final/broadcast [.,512] x2 = 2.
"""

import os
import sys

for _p in ("/opt/trn_rl_repo", "/root/.axon_site/_ro/trn_rl_repo"):
    if os.path.isdir(_p) and _p not in sys.path:
        sys.path.append(_p)

import math
from collections import deque

import numpy as np
import ml_dtypes

import concourse.bass as bass
import concourse.mybir as mybir
import concourse.tile as tile
from concourse import bacc, bass_utils

F32 = mybir.dt.float32
F32R = mybir.dt.float32r
BF16 = mybir.dt.bfloat16
AFT = mybir.ActivationFunctionType

P = 128
D = 2048
HD = 64
NJ = HD // 2          # 32 rope freqs
OQ = 512              # q-head dims per core (8 heads * 64)
OKV = 128             # kv-head dims per core (2 heads * 64)
NPAIR = 4             # head pairs per core
DT = D // P           # 16 d-tiles

HEAD_PERM = [0, 4, 1, 5, 2, 6, 3, 7]


def _emit_rope(nc, out_sb, in_ap, cos_ap, sin_ap, nh, tmp_pool):
    """RoPE: out[.., 2j] = x0*c - x1*s ; out[.., 2j+1] = x0*s + x1*c.
    in_ap: [128, nh*64] (PSUM f32); out_sb: [128, nh*64] (SBUF bf16);
    cos_ap/sin_ap: [128, 32] (per s-tile)."""
    w = nh * NJ
    x = in_ap.rearrange("p (h j t) -> p h j t", h=nh, j=NJ, t=2)
    o = out_sb.rearrange("p (h j t) -> p h j t", h=nh, j=NJ, t=2)
    x0, x1 = x[:, :, :, 0], x[:, :, :, 1]
    o0, o1 = o[:, :, :, 0], o[:, :, :, 1]
    c = cos_ap.unsqueeze(1).broadcast_to([P, nh, NJ])
    s = sin_ap.unsqueeze(1).broadcast_to([P, nh, NJ])
    ta = tmp_pool.tile([P, w], F32, tag="rope_ta")
    tb = tmp_pool.tile([P, w], F32, tag="rope_tb")
    ta3 = ta.rearrange("p (h j) -> p h j", h=nh, j=NJ)
    tb3 = tb.rearrange("p (h j) -> p h j", h=nh, j=NJ)
    nc.vector.tensor_mul(ta3, x0, c)
    nc.vector.tensor_mul(tb3, x1, s)
    nc.vector.tensor_sub(o0, ta3, tb3)
    nc.vector.tensor_mul(ta3, x0, s)
    nc.vector.tensor_mul(tb3, x1, c)
    nc.vector.tensor_add(o1, ta3, tb3)


def emit_kernel(nc, tc, ctx, S):
    NSC = S // 512        # s-chunks
    NST = S // P          # s-tiles (global)

    xT_d = nc.dram_tensor("xT", [D, S], BF16, kind="ExternalInput").ap()
    wqT_d = nc.dram_tensor("wqT", [D, OQ], BF16, kind="ExternalInput").ap()
    wkvT_d = nc.dram_tensor("wkvT", [D, 256], BF16, kind="ExternalInput").ap()
    woT_d = nc.dram_tensor("woT", [OQ, D], BF16, kind="ExternalInput").ap()
    cos_d = nc.dram_tensor("cost", [S, NJ], F32, kind="ExternalInput").ap()
    sin_d = nc.dram_tensor("sint", [S, NJ], F32, kind="ExternalInput").ap()
    out_d = nc.dram_tensor("out", [S, D], BF16, kind="ExternalOutput").ap()

    ctx.enter_context(nc.allow_low_precision(reason="bf16/f32r matmuls"))
    const = ctx.enter_context(tc.tile_pool(name="const", bufs=1))
    work = ctx.enter_context(tc.tile_pool(name="work", bufs=2))
    epool = ctx.enter_context(tc.tile_pool(name="epool", bufs=8))
    xTp = ctx.enter_context(tc.tile_pool(name="xTp", bufs=5))
    qTp = ctx.enter_context(tc.tile_pool(name="qTp", bufs=2))
    atp = ctx.enter_context(tc.tile_pool(name="atp", bufs=2))
    pvp = ctx.enter_context(tc.tile_pool(name="pvp", bufs=8))
    zgp = ctx.enter_context(tc.tile_pool(name="zgp", bufs=2))
    psS = ctx.enter_context(tc.tile_pool(name="psS", bufs=2, space="PSUM"))
    psV = ctx.enter_context(tc.tile_pool(name="psV", bufs=1, space="PSUM"))
    psF = ctx.enter_context(tc.tile_pool(name="psF", bufs=2, space="PSUM"))

    ones_f = const.tile([P, 1], F32)
    nc.any.memset(ones_f[:], 1.0)
    ones_r = const.tile([1, HD], F32R)
    nc.vector.tensor_copy(ones_r[:], ones_f[0:1, 0:1].broadcast_to([1, HD]))

    wqT = const.tile([P, DT * OQ], BF16)    # [d_loc, dt*512 + o']
    wkvT = const.tile([P, DT * 256], BF16)  # [d_loc, dt*256 + (k:0-127 | v:128-255)]
    woT = const.tile([P, NPAIR * D], BF16)  # [o'_loc, p*2048 + d]
    kT = const.tile([P, S], BF16)           # [o_kv, s]
    v2 = const.tile([P, NST * 130], BF16)   # [s_loc, g*130 + a*65 + (hd|one)]
    cosr = const.tile([P, NST * NJ], F32)
    sinr = const.tile([P, NST * NJ], F32)

    def emit_xt_load(c, st):
        g = c * 4 + st
        xT = xTp.tile([P, DT * P], BF16, tag="xT", name=f"xT_{g}")
        nc.sync.dma_start(xT[:].rearrange("p (dt s) -> p dt s", dt=DT, s=P),
                          xT_d[:, g * P:(g + 1) * P].rearrange("(dt p) s -> p dt s", p=P))
        return xT

    # ---- weight + x loads, ordered so the first projections start early and
    # wo (first needed during chunk 1) trails xT of chunks 0 and 1/st0 ----
    wq4 = wqT[:].rearrange("p (c4 dt o) -> p c4 dt o", c4=4, dt=4, o=OQ)
    wq4_d = wqT_d.rearrange("(c4 dt p) o -> p c4 dt o", c4=4, p=P)
    wk2 = wkvT[:].rearrange("p (c2 dt o) -> p c2 dt o", c2=2, dt=8, o=256)
    wk2_d = wkvT_d.rearrange("(c2 dt p) o -> p c2 dt o", c2=2, p=P)
    nc.sync.dma_start(wq4[:, 0], wq4_d[:, 0])
    xts0 = [emit_xt_load(0, 0)]
    nc.sync.dma_start(cosr[:].rearrange("p (g j) -> p g j", g=NST, j=NJ),
                      cos_d.rearrange("(g p) j -> p g j", p=P))
    nc.sync.dma_start(sinr[:].rearrange("p (g j) -> p g j", g=NST, j=NJ),
                      sin_d.rearrange("(g p) j -> p g j", p=P))
    nc.sync.dma_start(wk2[:, 0], wk2_d[:, 0])
    nc.sync.dma_start(wq4[:, 1], wq4_d[:, 1])
    xts0.append(emit_xt_load(0, 1))
    nc.sync.dma_start(wk2[:, 1], wk2_d[:, 1])
    nc.sync.dma_start(wq4[:, 2], wq4_d[:, 2])
    xts0.append(emit_xt_load(0, 2))
    nc.sync.dma_start(wq4[:, 3], wq4_d[:, 3])
    xts0.append(emit_xt_load(0, 3))
    xt_10 = emit_xt_load(1, 0)
    wo4 = woT[:].rearrange("p (pp d) -> p pp d", pp=NPAIR, d=D)
    wo4_d = woT_d.rearrange("(pp o) d -> o pp d", o=P)
    for i in range(NPAIR):
        nc.sync.dma_start(wo4[:, i], wo4_d[:, i])

    # ones columns of v2 (positions i*65 + 64)
    v2ones = v2[:].rearrange("p (i c) -> p i c", i=2 * NST, c=65)[:, :, 64]
    nc.vector.tensor_copy(v2ones, ones_f[:, 0:1].broadcast_to([P, 2 * NST]))

    # ---- projection quanta: each quantum is one PE matmul; rope / v-copy /
    # DMA-xbar transposes ride along with the last matmul they depend on ----
    def proj_quanta(c, qT, st, xts):
        g = c * 4 + st
        cos_ap = cosr[:, g * NJ:(g + 1) * NJ]
        sin_ap = sinr[:, g * NJ:(g + 1) * NJ]
        hold = {}

        def mk_q(dt):
            def f():
                if dt == 0:
                    if st + 1 < 4 and (st + 1) not in xts:
                        xts[st + 1] = emit_xt_load(c, st + 1)
                    hold['qp'] = psF.tile([P, OQ], F32, tag="fr", name=f"qp_{g}")
                nc.tensor.matmul(hold['qp'][:], xts[st][:, dt * P:(dt + 1) * P],
                                 wqT[:, dt * OQ:(dt + 1) * OQ],
                                 start=(dt == 0), stop=(dt == DT - 1),
                                 skip_group_check=True)
                if dt == DT - 1:
                    qr = work.tile([P, OQ], BF16, tag="qr")
                    _emit_rope(nc, qr[:], hold['qp'][:], cos_ap, sin_ap, 8, work)
                    for pa in range(NPAIR):
                        nc.sync.dma_start_transpose(
                            qT[:, pa * 512 + st * P: pa * 512 + (st + 1) * P],
                            qr[:, pa * P:(pa + 1) * P])
            return f

        def mk_kv(dt):
            def f():
                if dt == 0:
                    hold['kvp'] = psF.tile([P, 256], F32, tag="fr", name=f"kvp_{g}")
                nc.tensor.matmul(hold['kvp'][:], xts[st][:, dt * P:(dt + 1) * P],
                                 wkvT[:, dt * 256:(dt + 1) * 256],
                                 start=(dt == 0), stop=(dt == DT - 1),
                                 skip_group_check=True)
                if dt == DT - 1:
                    kvp = hold['kvp']
                    kr = work.tile([P, OKV], BF16, tag="kr")
                    _emit_rope(nc, kr[:], kvp[:, 0:OKV], cos_ap, sin_ap, 2, work)
                    v_src = kvp[:, OKV:256].rearrange("p (a x) -> p a x", a=2, x=HD)
                    v_dst = v2[:, g * 130:(g + 1) * 130].rearrange(
                        "p (a x) -> p a x", a=2, x=65)[:, :, 0:HD]
                    nc.vector.tensor_copy(v_dst, v_src)
                    nc.sync.dma_start_transpose(kT[:, g * P:(g + 1) * P], kr[:])
            return f

        return [mk_q(dt) for dt in range(DT)] + [mk_kv(dt) for dt in range(DT)]

    # ---- deferred normalization quanta for chunk c: Zinv broadcast (K=1
    # f32r matmul) + cast + multiply per (pair, half); ln/exp are emitted
    # immediately by the caller (ACT-only, off the PE queue) ----
    def norm_quanta(zi, pvs_list, attnT):
        def mk(p, half):
            def f():
                pvs = pvs_list[p]
                bc = psF.tile([HD, 512], F32, tag="fr")
                nc.tensor.matmul(bc[:], ones_r[:],
                                 zi[0:1, p * 1024 + half * 512:
                                    p * 1024 + (half + 1) * 512],
                                 skip_group_check=True)
                bcs = work.tile([HD, 512], BF16, tag="bc")
                nc.vector.tensor_copy(bcs[:], bc[:])
                if half == 0:
                    nc.vector.tensor_mul(attnT[0:HD, p * 512:(p + 1) * 512],
                                         pvs[0:HD, 0:512], bcs[:])
                else:
                    tmpb = work.tile([HD, 512], BF16, tag="tmpb", bufs=4)
                    nc.vector.tensor_mul(tmpb[:], pvs[0:HD, 512:1024], bcs[:])
                    # partition shift 0:64 -> 64:128 via sbuf-sbuf DMA
                    nc.sync.dma_start(attnT[HD:P, p * 512:(p + 1) * 512], tmpb[:])
            return f
        return [mk(p, h) for p in range(NPAIR) for h in range(2)]

    def emit_norm_prep(zg):
        """Batched softmax denominators: one Ln + one Exp for all 4 pairs.
        ln+exp live in one ACT table set with the scores' exp -> no reloads.
        The [4,1024] Zinv is DMA-flattened onto partition 0 because matmul
        rhs base partitions must be one of {0, 32, 64}."""
        zl = work.tile([4, 1024], F32, tag="zl")
        nc.scalar.activation(zl[:], zg[:], AFT.Ln)
        zi4 = work.tile([4, 1024], F32R, tag="zi4")
        nc.scalar.activation(zi4[:], zl[:], AFT.Exp, scale=-1.0)
        zi = work.tile([1, 4096], F32R, tag="zi")
        for p in range(NPAIR):
            nc.sync.dma_start(zi[0:1, p * 1024:(p + 1) * 1024], zi4[p:p + 1, :])
        return zi

    # ---- final matmul quanta: one PE matmul each; evacuate + output DMA
    # ride with the last accumulating matmul ----
    def final_quanta(c, attnT, st, tail=False):
        hold = {}

        def mk(dc, p):
            def f():
                if p == 0:
                    hold[dc] = psF.tile([P, 512], F32, tag="fr", name=f"rp_{c}_{st}_{dc}")
                rp = hold[dc]
                nc.tensor.matmul(rp[:], attnT[:, p * 512 + st * P: p * 512 + (st + 1) * P],
                                 woT[:, p * D + dc * 512: p * D + (dc + 1) * 512],
                                 start=(p == 0), stop=(p == NPAIR - 1),
                                 skip_group_check=True)
                if p == NPAIR - 1:
                    rs = work.tile([P, 512], BF16, tag="rs")
                    if tail and dc % 2 == 0:
                        # drain phase: ACT is idle, split evacuations
                        nc.scalar.copy(rs[:], rp[:])
                    else:
                        nc.vector.tensor_copy(rs[:], rp[:])
                    nc.sync.dma_start(out_d[(c * 4 + st) * P:(c * 4 + st + 1) * P,
                                            dc * 512:(dc + 1) * 512], rs[:])
            return f
        return [mk(dc, p) for dc in range(4) for p in range(NPAIR)]

    # ---- attention for one head pair; `pull` injects filler quanta between
    # the exp and the PV matmuls so the PE never idles on ACT latency ----
    def emit_attn_pair(c, p, qT, pull):
        NJT = 4 * (c + 1)
        pv = psV.tile([65, 1024], F32, tag="pv", name=f"pv_{c}_{p}")
        # j runs high-to-low so the diagonal tiles (whose e2 takes an extra
        # gpsimd affine_select hop after exp) come first: the unit then ends
        # on a full tile, keeping the gpsimd wake latency out of the
        # last-PV -> evacuate chain.
        for idx, j in enumerate(range(NJT - 1, -1, -1)):
            # causal: only columns q >= j*128 - c*512 within the chunk are live
            vs = max(0, (j - 4 * c) * P)
            w = 512 - vs
            sc2 = psS.tile([P, 1024], F32, tag="sc")
            nc.tensor.matmul(sc2[:, vs:512], kT[0:HD, j * P:(j + 1) * P],
                             qT[0:HD, p * 512 + vs:(p + 1) * 512])
            nc.tensor.matmul(sc2[:, 512 + vs:1024], kT[HD:P, j * P:(j + 1) * P],
                             qT[HD:P, p * 512 + vs:(p + 1) * 512])
            e2 = epool.tile([P, 1024], BF16, tag="e")
            e_v = e2[:].rearrange("p (h q) -> p h q", h=2, q=512)[:, :, vs:512]
            if vs:
                sc_v = sc2[:].rearrange("p (h q) -> p h q", h=2, q=512)[:, :, vs:512]
                nc.scalar.activation(e_v, sc_v, AFT.Exp, scale=1.0 / 8.0)
            else:
                nc.scalar.activation(e2[:], sc2[:], AFT.Exp, scale=1.0 / 8.0)
            if j >= 4 * c:  # diagonal block: zero where k_glob > q_glob
                # one merged op over both head-halves: iota resets per half
                nc.gpsimd.affine_select(
                    out=e_v, in_=e_v,
                    compare_op=mybir.AluOpType.is_ge, fill=0.0,
                    base=c * 512 + vs - j * P, channel_multiplier=-1,
                    pattern=[[0, 2], [1, w]])
            pull()
            nc.tensor.matmul(pv[:, vs:512], v2[:, j * 130: j * 130 + 65],
                             e2[:, vs:512],
                             start=(idx == 0), stop=(idx == NJT - 1), skip_group_check=True)
            nc.tensor.matmul(pv[:, 512 + vs:1024],
                             v2[:, j * 130 + 65: (j + 1) * 130],
                             e2[:, 512 + vs:1024],
                             start=(idx == 0), stop=(idx == NJT - 1), skip_group_check=True)
        # evacuate the accumulator to SBUF immediately so the single PV PSUM
        # slot frees for the next pair's j-loop. bf16 so downstream multiplies
        # hit the DVE 2x perf mode.
        pvs = pvp.tile([65, 1024], BF16, tag="pvs", name=f"pvs_{c}_{p}")
        nc.vector.tensor_copy(pvs[:], pv[:])
        return pvs

    # ---- main loop: chunk c's attention is interleaved (at single-matmul
    # granularity) with proj(c+1), final(c-1) and normalize(c-1) quanta ----
    qT_cur = qTp.tile([P, NPAIR * 512], BF16, tag="qT", name="qT_0")
    xts0_map = dict(enumerate(xts0))
    for st in range(4):
        for q in proj_quanta(0, qT_cur, st, xts0_map):
            q()

    pvs_prev = None     # chunk c-1 pair accumulators (unnormalized, bf16)
    zg_prev = None      # chunk c-1 Z rows [4, 1024]
    for c in range(NSC):
        qT = qT_cur
        items = []
        attnT = None
        if c >= 1:
            zi = emit_norm_prep(zg_prev)
            attnT = atp.tile([P, NPAIR * 512], BF16, tag="attnT", name=f"attnT_{c-1}")
            items += norm_quanta(zi, pvs_prev, attnT)
        if c + 1 < NSC:
            qT_cur = qTp.tile([P, NPAIR * 512], BF16, tag="qT", name=f"qT_{c+1}")
            xts = {0: xt_10} if c == 0 else {0: emit_xt_load(c + 1, 0)}
            pq = [proj_quanta(c + 1, qT_cur, st, xts) for st in range(4)]
        for st in range(4):
            if c + 1 < NSC:
                items += pq[st]
            if c >= 1:
                items += final_quanta(c - 1, attnT, st)
        filler = deque(items)
        steps = [4 * (c + 1) * NPAIR]  # j-steps left in this chunk

        def pull():
            n = (len(filler) + steps[0] - 1) // steps[0]
            steps[0] -= 1
            for _ in range(n):
                if filler:
                    filler.popleft()()

        zg = zgp.tile([4, 1024], BF16, tag="zg", name=f"zg_{c}")
        pvs_list = []
        for p in range(NPAIR):
            pvs = emit_attn_pair(c, p, qT, pull)
            nc.sync.dma_start(zg[p:p + 1, :], pvs[64:65, :])
            pvs_list.append(pvs)
        while filler:
            filler.popleft()()
        pvs_prev, zg_prev = pvs_list, zg

    # tail: normalize + final for the last chunk
    zi = emit_norm_prep(zg_prev)
    attnT = atp.tile([P, NPAIR * 512], BF16, tag="attnT", name=f"attnT_{NSC-1}")
    for q in norm_quanta(zi, pvs_prev, attnT):
        q()
    for st in range(4):
        for q in final_quanta(NSC - 1, attnT, st, tail=True):
            q()


_NC_CACHE = {}


def _pin_exp_ln_table_set():
    """Make the ACT-table-load pass resolve both Exp and Ln to the one set
    that contains them both (natural_log_exp_and_others). The default
    first-containing-set choice alternates exp_and_others / natural_log per
    activation, inserting a ~1.3us table reload before every softmax
    normalization. Only the advertised membership used for set *selection*
    is filtered; set indices stay canonical, so the runtime tables match."""
    if getattr(bacc, "_exp_ln_pinned", False):
        return
    real = bacc.get_activation_tables

    def pinned(arch):
        tables = dict(real(arch))
        both = {AFT.Exp, AFT.Ln}
        for name in list(tables):
            if name != "natural_log_exp_and_others" and (tables[name] & both):
                tables[name] = tables[name] - both
        return tables

    bacc.get_activation_tables = pinned
    bacc._exp_ln_pinned = True


def build(S=2048):
    if S in _NC_CACHE:
        return _NC_CACHE[S]
    from contextlib import ExitStack
    _pin_exp_ln_table_set()
    nc = bacc.Bacc("TRN2", target_bir_lowering=False, debug=False, num_devices=8)
    with tile.TileContext(nc) as tc, ExitStack() as ctx:
        emit_kernel(nc, tc, ctx, S)
    nc.compile()
    _NC_CACHE[S] = nc
    return nc


def shard_inputs(x, theta, wq, wk, wv, wo, S=2048):
    """Returns in_maps for 8 cores: core = b*4 + g. Pure layout prep."""
    cost = np.cos(theta[:S]).astype(np.float32)
    sint = np.sin(theta[:S]).astype(np.float32)
    in_maps = []
    for core in range(8):
        b, g = core // 4, core % 4
        wq_g = wq[g * 512:(g + 1) * 512].reshape(8, HD, D)[HEAD_PERM].reshape(512, D)
        wo_g = wo[:, g * 512:(g + 1) * 512].reshape(D, 8, HD)[:, HEAD_PERM].reshape(D, 512)
        wkv_g = np.concatenate([wk[g * 128:(g + 1) * 128], wv[g * 128:(g + 1) * 128]], axis=0)
        bf = ml_dtypes.bfloat16
        in_maps.append({
            "xT": np.ascontiguousarray(x[b, :S].T).astype(bf),
            "wqT": np.ascontiguousarray(wq_g.T).astype(bf),
            "wkvT": np.ascontiguousarray(wkv_g.T).astype(bf),
            "woT": np.ascontiguousarray(wo_g.T).astype(bf),
            "cost": cost,
            "sint": sint,
        })
    return in_maps


def run_on_hw(inputs, S=2048, trace=False):
    nc = build(S)
    in_maps = shard_inputs(inputs["x"], inputs["theta"], inputs["wq"],
                           inputs["wk"], inputs["wv"], inputs["wo"], S=S)
    res = bass_utils.run_bass_kernel_spmd(nc, in_maps, core_ids=list(range(8)),
                                          trace=trace)
    parts = [res.results[c]["out"].astype(np.float32) for c in range(8)]
    out = np.stack([parts[0] + parts[1] + parts[2] + parts[3],
                    parts[4] + parts[5] + parts[6] + parts[7]], axis=0)
    return out, res


def kernel(x, theta, mask, wq, wk, wv, wo):
    out, _ = run_on_hw({"x": np.asarray(x, np.float32), "theta": np.asarray(theta, np.float32),
                        "wq": np.asarray(wq, np.float32), "wk": np.asarray(wk, np.float32),
                        "wv": np.asarray(wv, np.float32), "wo": np.asarray(wo, np.float32)})
    return out


# revision 10
# speedup vs baseline: 1.1218x; 1.1218x over previous
"""Trainium2 Bass kernel for GQA attention (B=2, S=2048, D=2048, H=32, KVH=8).

Sharding: 8 cores = 2 batches x 4 head-groups. Each core handles one batch and
8 q-heads / 2 kv-heads: wq/wk/wv column-parallel, wo row-parallel; the partial
wo products are summed on the host.

Host-side prep (pure layout, no math): inputs are sharded, head-permuted and
pre-transposed so every matmul operand DMAs straight into its [K-on-partition]
layout; cos/sin of the rope angles are also computed host-side (the ScalarE Sin
LUT only covers [-pi, pi]).

Per-core kernel (all matmuls bf16/f32r):
  - q/k/v projections computed with s on partitions ([s,o] layout) from the
    pre-transposed xT/wqT/wkvT, RoPE applied with strided DVE ops writing bf16,
    then q/k transposed to [o,s] with the DMA xbar (off the PE).
  - scores are computed transposed: scT[k,q] = kT.T @ qT per head; exp on ACT;
    causal handled by skipping fully-masked k-tiles + one merged affine_select
    per diagonal tile (both head-halves in one gpsimd op).
  - PV: per (pair-group, pair) one [65,1024] PSUM accumulator; lhsT =
    [v_head | ones] (M=65) so the softmax denominator accumulates in PSUM row
    64 alongside the output.
  - normalization is DEFERRED and BATCHED per chunk: each pair's Z row is
    DMA-gathered into a [4,1024] tile; one Ln + one Exp (sharing the scores'
    ACT table set -> no table reloads) produce Zinv for all 4 pairs; the
    Zinv partition-broadcasts (K=1 f32r matmuls) + multiplies are emitted as
    pull-able quanta into the NEXT chunk.
  - final: res[s,d] = sum_p attnT_p.T @ woT_p, accumulated over 4 o-blocks.

Scheduling: the PE queue executes in order, so exp (ACT) latency inside the
attention j-loops is hidden by FINE-GRAINED interleaving: the projection of
chunk c+1, the final matmuls of chunk c-1 and the deferred normalization of
chunk c-1 are chopped into single-matmul "quanta" kept in a deque, and a
paced number of quanta is pulled between the scores and PV matmuls of every
j-step. Head order within a core is permuted to [0,4,1,5,2,6,3,7] so each
128-partition block pairs head h (kv0) with h+4 (kv1), letting the K=64 score
matmuls row-pack two heads concurrently on the PE array.

PSUM budget (8 banks): scores [128,1024] x2 bufs = 4, PV [65,1024] x1 = 2,
shared proj/final/broadcast [.,512] x2 = 2.
"""

import os
import sys

for _p in ("/opt/trn_rl_repo", "/root/.axon_site/_ro/trn_rl_repo"):
    if os.path.isdir(_p) and _p not in sys.path:
        sys.path.append(_p)

import math
from collections import deque

import numpy as np
import ml_dtypes

import concourse.bass as bass
import concourse.mybir as mybir
import concourse.tile as tile
from concourse import bacc, bass_utils

F32 = mybir.dt.float32
F32R = mybir.dt.float32r
BF16 = mybir.dt.bfloat16
AFT = mybir.ActivationFunctionType

P = 128
D = 2048
HD = 64
NJ = HD // 2          # 32 rope freqs
OQ = 512              # q-head dims per core (8 heads * 64)
OKV = 128             # kv-head dims per core (2 heads * 64)
NPAIR = 4             # head pairs per core
DT = D // P           # 16 d-tiles

HEAD_PERM = [0, 4, 1, 5, 2, 6, 3, 7]


def _emit_rope(nc, out_sb, in_ap, cos_ap, sin_ap, nh, tmp_pool):
    """RoPE: out[.., 2j] = x0*c - x1*s ; out[.., 2j+1] = x0*s + x1*c.
    in_ap: [128, nh*64] (PSUM f32); out_sb: [128, nh*64] (SBUF bf16);
    cos_ap/sin_ap: [128, 32] (per s-tile)."""
    w = nh * NJ
    x = in_ap.rearrange("p (h j t) -> p h j t", h=nh, j=NJ, t=2)
    o = out_sb.rearrange("p (h j t) -> p h j t", h=nh, j=NJ, t=2)
    x0, x1 = x[:, :, :, 0], x[:, :, :, 1]
    o0, o1 = o[:, :, :, 0], o[:, :, :, 1]
    c = cos_ap.unsqueeze(1).broadcast_to([P, nh, NJ])
    s = sin_ap.unsqueeze(1).broadcast_to([P, nh, NJ])
    ta = tmp_pool.tile([P, w], F32, tag="rope_ta")
    tb = tmp_pool.tile([P, w], F32, tag="rope_tb")
    ta3 = ta.rearrange("p (h j) -> p h j", h=nh, j=NJ)
    tb3 = tb.rearrange("p (h j) -> p h j", h=nh, j=NJ)
    nc.vector.tensor_mul(ta3, x0, c)
    nc.vector.tensor_mul(tb3, x1, s)
    nc.vector.tensor_sub(o0, ta3, tb3)
    nc.vector.tensor_mul(ta3, x0, s)
    nc.vector.tensor_mul(tb3, x1, c)
    nc.vector.tensor_add(o1, ta3, tb3)


def emit_kernel(nc, tc, ctx, S):
    NSC = S // 512        # s-chunks
    NST = S // P          # s-tiles (global)

    xT_d = nc.dram_tensor("xT", [D, S], BF16, kind="ExternalInput").ap()
    wqT_d = nc.dram_tensor("wqT", [D, OQ], BF16, kind="ExternalInput").ap()
    wkvT_d = nc.dram_tensor("wkvT", [D, 256], BF16, kind="ExternalInput").ap()
    woT_d = nc.dram_tensor("woT", [OQ, D], BF16, kind="ExternalInput").ap()
    cos_d = nc.dram_tensor("cost", [S, NJ], F32, kind="ExternalInput").ap()
    sin_d = nc.dram_tensor("sint", [S, NJ], F32, kind="ExternalInput").ap()
    out_d = nc.dram_tensor("out", [S, D], BF16, kind="ExternalOutput").ap()

    ctx.enter_context(nc.allow_low_precision(reason="bf16/f32r matmuls"))
    const = ctx.enter_context(tc.tile_pool(name="const", bufs=1))
    work = ctx.enter_context(tc.tile_pool(name="work", bufs=2))
    epool = ctx.enter_context(tc.tile_pool(name="epool", bufs=8))
    xTp = ctx.enter_context(tc.tile_pool(name="xTp", bufs=5))
    qTp = ctx.enter_context(tc.tile_pool(name="qTp", bufs=2))
    atp = ctx.enter_context(tc.tile_pool(name="atp", bufs=2))
    pvp = ctx.enter_context(tc.tile_pool(name="pvp", bufs=8))
    zgp = ctx.enter_context(tc.tile_pool(name="zgp", bufs=2))
    psS = ctx.enter_context(tc.tile_pool(name="psS", bufs=2, space="PSUM"))
    psV = ctx.enter_context(tc.tile_pool(name="psV", bufs=1, space="PSUM"))
    psF = ctx.enter_context(tc.tile_pool(name="psF", bufs=2, space="PSUM"))

    ones_f = const.tile([P, 1], F32)
    nc.any.memset(ones_f[:], 1.0)
    ones_r = const.tile([1, HD], F32R)
    nc.vector.tensor_copy(ones_r[:], ones_f[0:1, 0:1].broadcast_to([1, HD]))

    wqT = const.tile([P, DT * OQ], BF16)    # [d_loc, dt*512 + o']
    wkvT = const.tile([P, DT * 256], BF16)  # [d_loc, dt*256 + (k:0-127 | v:128-255)]
    woT = const.tile([P, NPAIR * D], BF16)  # [o'_loc, p*2048 + d]
    kT = const.tile([P, S], BF16)           # [o_kv, s]
    v2 = const.tile([P, NST * 130], BF16)   # [s_loc, g*130 + a*65 + (hd|one)]
    cosr = const.tile([P, NST * NJ], F32)
    sinr = const.tile([P, NST * NJ], F32)

    def emit_xt_load(c, st):
        g = c * 4 + st
        xT = xTp.tile([P, DT * P], BF16, tag="xT", name=f"xT_{g}")
        nc.sync.dma_start(xT[:].rearrange("p (dt s) -> p dt s", dt=DT, s=P),
                          xT_d[:, g * P:(g + 1) * P].rearrange("(dt p) s -> p dt s", p=P))
        return xT

    # ---- weight + x loads, ordered so the first projections start early and
    # wo (first needed during chunk 1) trails xT of chunks 0 and 1/st0 ----
    wq4 = wqT[:].rearrange("p (c4 dt o) -> p c4 dt o", c4=4, dt=4, o=OQ)
    wq4_d = wqT_d.rearrange("(c4 dt p) o -> p c4 dt o", c4=4, p=P)
    wk2 = wkvT[:].rearrange("p (c2 dt o) -> p c2 dt o", c2=2, dt=8, o=256)
    wk2_d = wkvT_d.rearrange("(c2 dt p) o -> p c2 dt o", c2=2, p=P)
    nc.sync.dma_start(wq4[:, 0], wq4_d[:, 0])
    xts0 = [emit_xt_load(0, 0)]
    nc.sync.dma_start(cosr[:].rearrange("p (g j) -> p g j", g=NST, j=NJ),
                      cos_d.rearrange("(g p) j -> p g j", p=P))
    nc.sync.dma_start(sinr[:].rearrange("p (g j) -> p g j", g=NST, j=NJ),
                      sin_d.rearrange("(g p) j -> p g j", p=P))
    nc.sync.dma_start(wk2[:, 0], wk2_d[:, 0])
    nc.sync.dma_start(wq4[:, 1], wq4_d[:, 1])
    nc.sync.dma_start(wq4[:, 2], wq4_d[:, 2])
    nc.sync.dma_start(wq4[:, 3], wq4_d[:, 3])
    nc.sync.dma_start(wk2[:, 1], wk2_d[:, 1])
    xts0.append(emit_xt_load(0, 1))
    xts0.append(emit_xt_load(0, 2))
    xts0.append(emit_xt_load(0, 3))
    xt_10 = emit_xt_load(1, 0)
    wo4 = woT[:].rearrange("p (pp d) -> p pp d", pp=NPAIR, d=D)
    wo4_d = woT_d.rearrange("(pp o) d -> o pp d", o=P)
    for i in range(NPAIR):
        nc.sync.dma_start(wo4[:, i], wo4_d[:, i])

    # ones columns of v2 (positions i*65 + 64)
    v2ones = v2[:].rearrange("p (i c) -> p i c", i=2 * NST, c=65)[:, :, 64]
    nc.vector.tensor_copy(v2ones, ones_f[:, 0:1].broadcast_to([P, 2 * NST]))

    # ---- projection quanta: each quantum is one PE matmul; rope / v-copy /
    # DMA-xbar transposes ride along with the last matmul they depend on ----
    def proj_quanta(c, qT, st, xts):
        g = c * 4 + st
        cos_ap = cosr[:, g * NJ:(g + 1) * NJ]
        sin_ap = sinr[:, g * NJ:(g + 1) * NJ]
        hold = {}

        def mk_q(dt):
            def f():
                if dt == 0:
                    if st + 1 < 4 and (st + 1) not in xts:
                        xts[st + 1] = emit_xt_load(c, st + 1)
                    hold['qp'] = psF.tile([P, OQ], F32, tag="fr", name=f"qp_{g}")
                nc.tensor.matmul(hold['qp'][:], xts[st][:, dt * P:(dt + 1) * P],
                                 wqT[:, dt * OQ:(dt + 1) * OQ],
                                 start=(dt == 0), stop=(dt == DT - 1),
                                 skip_group_check=True)
                if dt == DT - 1:
                    qr = work.tile([P, OQ], BF16, tag="qr")
                    _emit_rope(nc, qr[:], hold['qp'][:], cos_ap, sin_ap, 8, work)
                    hold['qr'] = qr
            return f

        def mk_kv(dt):
            def f():
                if dt == 0:
                    hold['kvp'] = psF.tile([P, 256], F32, tag="fr", name=f"kvp_{g}")
                nc.tensor.matmul(hold['kvp'][:], xts[st][:, dt * P:(dt + 1) * P],
                                 wkvT[:, dt * 256:(dt + 1) * 256],
                                 start=(dt == 0), stop=(dt == DT - 1),
                                 skip_group_check=True)
                if dt == DT - 1:
                    kvp = hold['kvp']
                    kr = work.tile([P, OKV], BF16, tag="kr")
                    _emit_rope(nc, kr[:], kvp[:, 0:OKV], cos_ap, sin_ap, 2, work)
                    v_src = kvp[:, OKV:256].rearrange("p (a x) -> p a x", a=2, x=HD)
                    v_dst = v2[:, g * 130:(g + 1) * 130].rearrange(
                        "p (a x) -> p a x", a=2, x=65)[:, :, 0:HD]
                    nc.vector.tensor_copy(v_dst, v_src)
                    hold['kr'] = kr
            return f

        def mk_tp():
            # one batched DMA-xbar issue transposes all 4 q head-pair blocks
            # ([128,512] -> [128,4,128]); trailing the rope by a full dt-loop
            # of quanta so the sync queue never head-of-line blocks on DVE
            nc.sync.dma_start_transpose(
                qT[:].rearrange("o (pa s) -> o pa s", pa=NPAIR, s=512)
                     [:, :, st * P:(st + 1) * P],
                hold['qr'][:])
            nc.sync.dma_start_transpose(kT[:, g * P:(g + 1) * P], hold['kr'][:])

        return ([mk_q(dt) for dt in range(DT)] + [mk_kv(dt) for dt in range(DT)]
                + [mk_tp])

    # ---- deferred normalization quanta for chunk c: Zinv broadcast (K=1
    # f32r matmul) + cast + multiply per (pair, half); ln/exp are emitted
    # immediately by the caller (ACT-only, off the PE queue) ----
    def norm_quanta(zi, pvs_list, attnT):
        def mk(p, half):
            def f():
                pvs = pvs_list[p]
                bc = psF.tile([HD, 512], F32, tag="fr")
                nc.tensor.matmul(bc[:], ones_r[:],
                                 zi[0:1, p * 1024 + half * 512:
                                    p * 1024 + (half + 1) * 512],
                                 skip_group_check=True)
                bcs = work.tile([HD, 512], BF16, tag="bc")
                nc.vector.tensor_copy(bcs[:], bc[:])
                if half == 0:
                    nc.vector.tensor_mul(attnT[0:HD, p * 512:(p + 1) * 512],
                                         pvs[0:HD, 0:512], bcs[:])
                else:
                    tmpb = work.tile([HD, 512], BF16, tag="tmpb", bufs=4)
                    nc.vector.tensor_mul(tmpb[:], pvs[0:HD, 512:1024], bcs[:])
                    # partition shift 0:64 -> 64:128 via sbuf-sbuf DMA
                    nc.sync.dma_start(attnT[HD:P, p * 512:(p + 1) * 512], tmpb[:])
            return f
        return [mk(p, h) for p in range(NPAIR) for h in range(2)]

    def emit_norm_prep(zg):
        """Batched softmax denominators: one Ln + one Exp for all 4 pairs.
        ln+exp live in one ACT table set with the scores' exp -> no reloads.
        The [4,1024] Zinv is DMA-flattened onto partition 0 because matmul
        rhs base partitions must be one of {0, 32, 64}."""
        zl = work.tile([4, 1024], F32, tag="zl")
        nc.scalar.activation(zl[:], zg[:], AFT.Ln)
        zi4 = work.tile([4, 1024], F32R, tag="zi4")
        nc.scalar.activation(zi4[:], zl[:], AFT.Exp, scale=-1.0)
        zi = work.tile([1, 4096], F32R, tag="zi")
        for p in range(NPAIR):
            nc.sync.dma_start(zi[0:1, p * 1024:(p + 1) * 1024], zi4[p:p + 1, :])
        return zi

    # ---- final matmul quanta: one PE matmul each; evacuate + output DMA
    # ride with the last accumulating matmul ----
    def final_quanta(c, attnT, st, tail=False):
        hold = {}

        def mk(dc, p):
            def f():
                if p == 0:
                    hold[dc] = psF.tile([P, 512], F32, tag="fr", name=f"rp_{c}_{st}_{dc}")
                rp = hold[dc]
                nc.tensor.matmul(rp[:], attnT[:, p * 512 + st * P: p * 512 + (st + 1) * P],
                                 woT[:, p * D + dc * 512: p * D + (dc + 1) * 512],
                                 start=(p == 0), stop=(p == NPAIR - 1),
                                 skip_group_check=True)
                if p == NPAIR - 1:
                    rs = work.tile([P, 512], BF16, tag="rs")
                    if tail and dc % 2 == 0:
                        # drain phase: ACT is idle, split evacuations
                        nc.scalar.copy(rs[:], rp[:])
                    else:
                        nc.vector.tensor_copy(rs[:], rp[:])
                    nc.sync.dma_start(out_d[(c * 4 + st) * P:(c * 4 + st + 1) * P,
                                            dc * 512:(dc + 1) * 512], rs[:])
            return f
        return [mk(dc, p) for dc in range(4) for p in range(NPAIR)]

    # ---- attention for one head pair; `pull` injects filler quanta between
    # the exp and the PV matmuls so the PE never idles on ACT latency ----
    def emit_attn_pair(c, p, qT, pull):
        NJT = 4 * (c + 1)
        pv = psV.tile([65, 1024], F32, tag="pv", name=f"pv_{c}_{p}")
        # j runs high-to-low so the diagonal tiles (whose e2 takes an extra
        # gpsimd affine_select hop after exp) come first: the unit then ends
        # on a full tile, keeping the gpsimd wake latency out of the
        # last-PV -> evacuate chain.
        for idx, j in enumerate(range(NJT - 1, -1, -1)):
            # causal: only columns q >= j*128 - c*512 within the chunk are live
            vs = max(0, (j - 4 * c) * P)
            w = 512 - vs
            sc2 = psS.tile([P, 1024], F32, tag="sc")
            nc.tensor.matmul(sc2[:, vs:512], kT[0:HD, j * P:(j + 1) * P],
                             qT[0:HD, p * 512 + vs:(p + 1) * 512])
            nc.tensor.matmul(sc2[:, 512 + vs:1024], kT[HD:P, j * P:(j + 1) * P],
                             qT[HD:P, p * 512 + vs:(p + 1) * 512])
            e2 = epool.tile([P, 1024], BF16, tag="e")
            e_v = e2[:].rearrange("p (h q) -> p h q", h=2, q=512)[:, :, vs:512]
            if vs:
                sc_v = sc2[:].rearrange("p (h q) -> p h q", h=2, q=512)[:, :, vs:512]
                nc.scalar.activation(e_v, sc_v, AFT.Exp, scale=1.0 / 8.0)
            else:
                nc.scalar.activation(e2[:], sc2[:], AFT.Exp, scale=1.0 / 8.0)
            if j >= 4 * c:  # diagonal block: zero where k_glob > q_glob
                # one merged op over both head-halves: iota resets per half
                nc.gpsimd.affine_select(
                    out=e_v, in_=e_v,
                    compare_op=mybir.AluOpType.is_ge, fill=0.0,
                    base=c * 512 + vs - j * P, channel_multiplier=-1,
                    pattern=[[0, 2], [1, w]])
            pull()
            nc.tensor.matmul(pv[:, vs:512], v2[:, j * 130: j * 130 + 65],
                             e2[:, vs:512],
                             start=(idx == 0), stop=(idx == NJT - 1), skip_group_check=True)
            nc.tensor.matmul(pv[:, 512 + vs:1024],
                             v2[:, j * 130 + 65: (j + 1) * 130],
                             e2[:, 512 + vs:1024],
                             start=(idx == 0), stop=(idx == NJT - 1), skip_group_check=True)
        # evacuate the accumulator to SBUF immediately so the single PV PSUM
        # slot frees for the next pair's j-loop. bf16 so downstream multiplies
        # hit the DVE 2x perf mode.
        pvs = pvp.tile([65, 1024], BF16, tag="pvs", name=f"pvs_{c}_{p}")
        nc.vector.tensor_copy(pvs[:], pv[:])
        return pvs

    # ---- main loop: chunk c's attention is interleaved (at single-matmul
    # granularity) with proj(c+1), final(c-1) and normalize(c-1) quanta ----
    qT_cur = qTp.tile([P, NPAIR * 512], BF16, tag="qT", name="qT_0")
    xts0_map = dict(enumerate(xts0))
    for st in range(4):
        for q in proj_quanta(0, qT_cur, st, xts0_map):
            q()

    pvs_prev = None     # chunk c-1 pair accumulators (unnormalized, bf16)
    zg_prev = None      # chunk c-1 Z rows [4, 1024]
    for c in range(NSC):
        qT = qT_cur
        items = []
        attnT = None
        if c >= 1:
            zi = emit_norm_prep(zg_prev)
            attnT = atp.tile([P, NPAIR * 512], BF16, tag="attnT", name=f"attnT_{c-1}")
            items += norm_quanta(zi, pvs_prev, attnT)
        if c + 1 < NSC:
            qT_cur = qTp.tile([P, NPAIR * 512], BF16, tag="qT", name=f"qT_{c+1}")
            xts = {0: xt_10} if c == 0 else {0: emit_xt_load(c + 1, 0)}
            pq = [proj_quanta(c + 1, qT_cur, st, xts) for st in range(4)]
        for st in range(4):
            if c + 1 < NSC:
                items += pq[st]
            if c >= 1:
                items += final_quanta(c - 1, attnT, st)
        filler = deque(items)
        steps = [4 * (c + 1) * NPAIR]  # j-steps left in this chunk

        def pull():
            n = (len(filler) + steps[0] - 1) // steps[0]
            steps[0] -= 1
            for _ in range(n):
                if filler:
                    filler.popleft()()

        zg = zgp.tile([4, 1024], BF16, tag="zg", name=f"zg_{c}")
        pvs_list = []
        for p in range(NPAIR):
            pvs = emit_attn_pair(c, p, qT, pull)
            nc.sync.dma_start(zg[p:p + 1, :], pvs[64:65, :])
            pvs_list.append(pvs)
        while filler:
            filler.popleft()()
        pvs_prev, zg_prev = pvs_list, zg

    # tail: normalize + final for the last chunk
    zi = emit_norm_prep(zg_prev)
    attnT = atp.tile([P, NPAIR * 512], BF16, tag="attnT", name=f"attnT_{NSC-1}")
    for q in norm_quanta(zi, pvs_prev, attnT):
        q()
    for st in range(4):
        for q in final_quanta(NSC - 1, attnT, st, tail=True):
            q()


_NC_CACHE = {}


def _pin_exp_ln_table_set():
    """Make the ACT-table-load pass resolve both Exp and Ln to the one set
    that contains them both (natural_log_exp_and_others). The default
    first-containing-set choice alternates exp_and_others / natural_log per
    activation, inserting a ~1.3us table reload before every softmax
    normalization. Only the advertised membership used for set *selection*
    is filtered; set indices stay canonical, so the runtime tables match."""
    if getattr(bacc, "_exp_ln_pinned", False):
        return
    real = bacc.get_activation_tables

    def pinned(arch):
        tables = dict(real(arch))
        both = {AFT.Exp, AFT.Ln}
        for name in list(tables):
            if name != "natural_log_exp_and_others" and (tables[name] & both):
                tables[name] = tables[name] - both
        return tables

    bacc.get_activation_tables = pinned
    bacc._exp_ln_pinned = True


def build(S=2048):
    if S in _NC_CACHE:
        return _NC_CACHE[S]
    from contextlib import ExitStack
    _pin_exp_ln_table_set()
    nc = bacc.Bacc("TRN2", target_bir_lowering=False, debug=False, num_devices=8)
    with tile.TileContext(nc) as tc, ExitStack() as ctx:
        emit_kernel(nc, tc, ctx, S)
    nc.compile()
    _NC_CACHE[S] = nc
    return nc


def shard_inputs(x, theta, wq, wk, wv, wo, S=2048):
    """Returns in_maps for 8 cores: core = b*4 + g. Pure layout prep."""
    cost = np.cos(theta[:S]).astype(np.float32)
    sint = np.sin(theta[:S]).astype(np.float32)
    in_maps = []
    for core in range(8):
        b, g = core // 4, core % 4
        wq_g = wq[g * 512:(g + 1) * 512].reshape(8, HD, D)[HEAD_PERM].reshape(512, D)
        wo_g = wo[:, g * 512:(g + 1) * 512].reshape(D, 8, HD)[:, HEAD_PERM].reshape(D, 512)
        wkv_g = np.concatenate([wk[g * 128:(g + 1) * 128], wv[g * 128:(g + 1) * 128]], axis=0)
        bf = ml_dtypes.bfloat16
        in_maps.append({
            "xT": np.ascontiguousarray(x[b, :S].T).astype(bf),
            "wqT": np.ascontiguousarray(wq_g.T).astype(bf),
            "wkvT": np.ascontiguousarray(wkv_g.T).astype(bf),
            "woT": np.ascontiguousarray(wo_g.T).astype(bf),
            "cost": cost,
            "sint": sint,
        })
    return in_maps


def run_on_hw(inputs, S=2048, trace=False):
    nc = build(S)
    in_maps = shard_inputs(inputs["x"], inputs["theta"], inputs["wq"],
                           inputs["wk"], inputs["wv"], inputs["wo"], S=S)
    res = bass_utils.run_bass_kernel_spmd(nc, in_maps, core_ids=list(range(8)),
                                          trace=trace)
    parts = [res.results[c]["out"].astype(np.float32) for c in range(8)]
    out = np.stack([parts[0] + parts[1] + parts[2] + parts[3],
                    parts[4] + parts[5] + parts[6] + parts[7]], axis=0)
    return out, res


def kernel(x, theta, mask, wq, wk, wv, wo):
    out, _ = run_on_hw({"x": np.asarray(x, np.float32), "theta": np.asarray(theta, np.float32),
                        "wq": np.asarray(wq, np.float32), "wk": np.asarray(wk, np.float32),
                        "wv": np.asarray(wv, np.float32), "wo": np.asarray(wo, np.float32)})
    return out


# revision 29
# speedup vs baseline: 1.1384x; 1.0148x over previous
"""Trainium2 Bass kernel for GQA attention (B=2, S=2048, D=2048, H=32, KVH=8).

Sharding: 8 cores = 2 batches x 4 head-groups. Each core handles one batch and
8 q-heads / 2 kv-heads: wq/wk/wv column-parallel, wo row-parallel; the partial
wo products are summed on the host.

Host-side prep (pure layout, no math): inputs are sharded, head-permuted and
pre-transposed so every matmul operand DMAs straight into its [K-on-partition]
layout; cos/sin of the rope angles are also computed host-side (the ScalarE Sin
LUT only covers [-pi, pi]).

Per-core kernel (all matmuls bf16/f32r):
  - q/k/v projections computed with s on partitions ([s,o] layout) from the
    pre-transposed xT/wqT/wkvT, RoPE applied with strided DVE ops writing bf16,
    then q/k transposed to [o,s] with the DMA xbar (off the PE).
  - scores are computed transposed: scT[k,q] = kT.T @ qT per head; exp on ACT;
    causal handled by skipping fully-masked k-tiles + one merged affine_select
    per diagonal tile (both head-halves in one gpsimd op).
  - PV: per (pair-group, pair) one [65,1024] PSUM accumulator; lhsT =
    [v_head | ones] (M=65) so the softmax denominator accumulates in PSUM row
    64 alongside the output.
  - normalization is DEFERRED and BATCHED per chunk: each pair's Z row is
    DMA-gathered into a [4,1024] tile; one Ln + one Exp (sharing the scores'
    ACT table set -> no table reloads) produce Zinv for all 4 pairs; the
    Zinv partition-broadcasts (K=1 f32r matmuls) + multiplies are emitted as
    pull-able quanta into the NEXT chunk.
  - final: res[s,d] = sum_p attnT_p.T @ woT_p, accumulated over 4 o-blocks.

Scheduling: the PE queue executes in order, so exp (ACT) latency inside the
attention j-loops is hidden by FINE-GRAINED interleaving: the projection of
chunk c+1, the final matmuls of chunk c-1 and the deferred normalization of
chunk c-1 are chopped into single-matmul "quanta" kept in a deque, and a
paced number of quanta is pulled between the scores and PV matmuls of every
j-step. Head order within a core is permuted to [0,4,1,5,2,6,3,7] so each
128-partition block pairs head h (kv0) with h+4 (kv1), letting the K=64 score
matmuls row-pack two heads concurrently on the PE array.

PSUM budget (8 banks): scores [128,1024] x2 bufs = 4, PV [65,1024] x1 = 2,
shared proj/final/broadcast [.,512] x2 = 2.
"""

import os
import sys

for _p in ("/opt/trn_rl_repo", "/root/.axon_site/_ro/trn_rl_repo"):
    if os.path.isdir(_p) and _p not in sys.path:
        sys.path.append(_p)

import math
from collections import deque

import numpy as np
import ml_dtypes

import concourse.bass as bass
import concourse.mybir as mybir
import concourse.tile as tile
from concourse import bacc, bass_utils

F32 = mybir.dt.float32
F32R = mybir.dt.float32r
BF16 = mybir.dt.bfloat16
AFT = mybir.ActivationFunctionType

P = 128
D = 2048
HD = 64
NJ = HD // 2          # 32 rope freqs
OQ = 512              # q-head dims per core (8 heads * 64)
OKV = 128             # kv-head dims per core (2 heads * 64)
NPAIR = 4             # head pairs per core
DT = D // P           # 16 d-tiles

HEAD_PERM = [0, 4, 1, 5, 2, 6, 3, 7]

DEBUG_DUMP = False   # emit extra DRAM outputs with last-chunk intermediates


def _emit_rope(nc, out_sb, in_ap, cos_ap, sin_ap, nh, tmp_pool):
    """RoPE: out[.., 2j] = x0*c - x1*s ; out[.., 2j+1] = x0*s + x1*c.
    in_ap: [128, nh*64] (PSUM f32); out_sb: [128, nh*64] (SBUF bf16);
    cos_ap/sin_ap: [128, 32] (per s-tile)."""
    w = nh * NJ
    x = in_ap.rearrange("p (h j t) -> p h j t", h=nh, j=NJ, t=2)
    o = out_sb.rearrange("p (h j t) -> p h j t", h=nh, j=NJ, t=2)
    x0, x1 = x[:, :, :, 0], x[:, :, :, 1]
    o0, o1 = o[:, :, :, 0], o[:, :, :, 1]
    c = cos_ap.unsqueeze(1).broadcast_to([P, nh, NJ])
    s = sin_ap.unsqueeze(1).broadcast_to([P, nh, NJ])
    ta = tmp_pool.tile([P, w], F32, tag="rope_ta")
    tb = tmp_pool.tile([P, w], F32, tag="rope_tb")
    ta3 = ta.rearrange("p (h j) -> p h j", h=nh, j=NJ)
    tb3 = tb.rearrange("p (h j) -> p h j", h=nh, j=NJ)
    nc.vector.tensor_mul(ta3, x0, c)
    nc.vector.tensor_mul(tb3, x1, s)
    nc.vector.tensor_sub(o0, ta3, tb3)
    nc.vector.tensor_mul(ta3, x0, s)
    nc.vector.tensor_mul(tb3, x1, c)
    nc.vector.tensor_add(o1, ta3, tb3)


def emit_kernel(nc, tc, ctx, S):
    NSC = S // 512        # s-chunks
    NST = S // P          # s-tiles (global)

    xT_d = nc.dram_tensor("xT", [D, S], BF16, kind="ExternalInput").ap()
    wqT_d = nc.dram_tensor("wqT", [D, OQ], BF16, kind="ExternalInput").ap()
    wkvT_d = nc.dram_tensor("wkvT", [D, 256], BF16, kind="ExternalInput").ap()
    woT_d = nc.dram_tensor("woT", [OQ, D], BF16, kind="ExternalInput").ap()
    cos_d = nc.dram_tensor("cost", [S, NJ], F32, kind="ExternalInput").ap()
    sin_d = nc.dram_tensor("sint", [S, NJ], F32, kind="ExternalInput").ap()
    out_d = nc.dram_tensor("out", [S, D], BF16, kind="ExternalOutput").ap()

    ctx.enter_context(nc.allow_low_precision(reason="bf16/f32r matmuls"))
    const = ctx.enter_context(tc.tile_pool(name="const", bufs=1))
    work = ctx.enter_context(tc.tile_pool(name="work", bufs=2))
    epool = ctx.enter_context(tc.tile_pool(name="epool", bufs=8))
    xTp = ctx.enter_context(tc.tile_pool(name="xTp", bufs=5))
    qTp = ctx.enter_context(tc.tile_pool(name="qTp", bufs=2))
    atp = ctx.enter_context(tc.tile_pool(name="atp", bufs=2))
    pvp = ctx.enter_context(tc.tile_pool(name="pvp", bufs=8))
    zgp = ctx.enter_context(tc.tile_pool(name="zgp", bufs=1))
    psS = ctx.enter_context(tc.tile_pool(name="psS", bufs=2, space="PSUM"))
    psV = ctx.enter_context(tc.tile_pool(name="psV", bufs=1, space="PSUM"))
    psF = ctx.enter_context(tc.tile_pool(name="psF", bufs=2, space="PSUM"))

    ones_f = const.tile([P, 1], F32)
    nc.any.memset(ones_f[:], 1.0)
    ones_r = const.tile([1, HD], F32R)
    nc.vector.tensor_copy(ones_r[:], ones_f[0:1, 0:1].broadcast_to([1, HD]))

    wqT = const.tile([P, DT * OQ], BF16)    # [d_loc, dt*512 + o']
    wkvT = const.tile([P, DT * 256], BF16)  # [d_loc, dt*256 + (k:0-127 | v:128-255)]
    woT = const.tile([P, NPAIR * D], BF16)  # [o'_loc, p*2048 + d]
    kT = const.tile([P, S], BF16)           # [o_kv, s]
    v2 = const.tile([P, NST * 130], BF16)   # [s_loc, g*130 + a*65 + (hd|one)]
    cosr = const.tile([P, NST * NJ], F32)
    sinr = const.tile([P, NST * NJ], F32)

    def emit_xt_load(c, st):
        g = c * 4 + st
        xT = xTp.tile([P, DT * P], BF16, tag="xT", name=f"xT_{g}")
        nc.sync.dma_start(xT[:].rearrange("p (dt s) -> p dt s", dt=DT, s=P),
                          xT_d[:, g * P:(g + 1) * P].rearrange("(dt p) s -> p dt s", p=P))
        return xT

    # ---- weight + x loads, ordered so the first projections start early and
    # wo (first needed during chunk 1) trails xT of chunks 0 and 1/st0 ----
    wq4 = wqT[:].rearrange("p (c4 dt o) -> p c4 dt o", c4=4, dt=4, o=OQ)
    wq4_d = wqT_d.rearrange("(c4 dt p) o -> p c4 dt o", c4=4, p=P)
    wk2 = wkvT[:].rearrange("p (c2 dt o) -> p c2 dt o", c2=2, dt=8, o=256)
    wk2_d = wkvT_d.rearrange("(c2 dt p) o -> p c2 dt o", c2=2, p=P)
    nc.sync.dma_start(wq4[:, 0], wq4_d[:, 0])
    xts0 = [emit_xt_load(0, 0)]
    nc.sync.dma_start(wq4[:, 1], wq4_d[:, 1])
    nc.sync.dma_start(cosr[:].rearrange("p (g j) -> p g j", g=NST, j=NJ),
                      cos_d.rearrange("(g p) j -> p g j", p=P))
    nc.sync.dma_start(sinr[:].rearrange("p (g j) -> p g j", g=NST, j=NJ),
                      sin_d.rearrange("(g p) j -> p g j", p=P))
    nc.sync.dma_start(wq4[:, 2], wq4_d[:, 2])
    nc.sync.dma_start(wq4[:, 3], wq4_d[:, 3])
    nc.sync.dma_start(wk2[:, 0], wk2_d[:, 0])
    nc.sync.dma_start(wk2[:, 1], wk2_d[:, 1])
    xts0.append(emit_xt_load(0, 1))
    xts0.append(emit_xt_load(0, 2))
    xts0.append(emit_xt_load(0, 3))
    xt_first = {1: emit_xt_load(1, 0)}
    wo4 = woT[:].rearrange("p (pp d) -> p pp d", pp=NPAIR, d=D)
    wo4_d = woT_d.rearrange("(pp o) d -> o pp d", o=P)
    for i in range(NPAIR):
        nc.sync.dma_start(wo4[:, i], wo4_d[:, i])

    # ones columns of v2 (positions i*65 + 64)
    v2ones = v2[:].rearrange("p (i c) -> p i c", i=2 * NST, c=65)[:, :, 64]
    nc.vector.tensor_copy(v2ones, ones_f[:, 0:1].broadcast_to([P, 2 * NST]))

    # ---- projection quanta: each quantum is one PE matmul; rope / v-copy /
    # DMA-xbar transposes ride along with the last matmul they depend on ----
    def proj_quanta(c, qT, st, xts):
        g = c * 4 + st
        cos_ap = cosr[:, g * NJ:(g + 1) * NJ]
        sin_ap = sinr[:, g * NJ:(g + 1) * NJ]
        hold = {}

        def mk_q(dt):
            def f():
                if dt == 0:
                    if st + 1 < 4 and (st + 1) not in xts:
                        xts[st + 1] = emit_xt_load(c, st + 1)
                    hold['qp'] = psF.tile([P, OQ], F32, tag="fr", name=f"qp_{g}")
                nc.tensor.matmul(hold['qp'][:], xts[st][:, dt * P:(dt + 1) * P],
                                 wqT[:, dt * OQ:(dt + 1) * OQ],
                                 start=(dt == 0), stop=(dt == DT - 1),
                                 skip_group_check=True)
                if dt == DT - 1:
                    qr = work.tile([P, OQ], BF16, tag="qr")
                    _emit_rope(nc, qr[:], hold['qp'][:], cos_ap, sin_ap, 8, work)
                    hold['qr'] = qr
            return f

        def mk_kv(dt):
            def f():
                if dt == 0:
                    hold['kvp'] = psF.tile([P, 256], F32, tag="fr", name=f"kvp_{g}")
                nc.tensor.matmul(hold['kvp'][:], xts[st][:, dt * P:(dt + 1) * P],
                                 wkvT[:, dt * 256:(dt + 1) * 256],
                                 start=(dt == 0), stop=(dt == DT - 1),
                                 skip_group_check=True)
                if dt == DT - 1:
                    kvp = hold['kvp']
                    kr = work.tile([P, OKV], BF16, tag="kr")
                    _emit_rope(nc, kr[:], kvp[:, 0:OKV], cos_ap, sin_ap, 2, work)
                    v_src = kvp[:, OKV:256].rearrange("p (a x) -> p a x", a=2, x=HD)
                    v_dst = v2[:, g * 130:(g + 1) * 130].rearrange(
                        "p (a x) -> p a x", a=2, x=65)[:, :, 0:HD]
                    nc.vector.tensor_copy(v_dst, v_src)
                    hold['kr'] = kr
            return f

        def mk_tp():
            # one batched DMA-xbar issue transposes all 4 q head-pair blocks
            # ([128,512] -> [128,4,128]); trailing the rope by a full dt-loop
            # of quanta so the sync queue never head-of-line blocks on DVE
            nc.sync.dma_start_transpose(
                qT[:].rearrange("o (pa s) -> o pa s", pa=NPAIR, s=512)
                     [:, :, st * P:(st + 1) * P],
                hold['qr'][:])
            nc.sync.dma_start_transpose(kT[:, g * P:(g + 1) * P], hold['kr'][:])
            if st == 3 and c + 1 < NSC and (c + 1) not in xt_first:
                # issue the NEXT chunk's first x load a full chunk early: if
                # it lands with the boundary, proj(c+1) st0's matmuls (pulled
                # at the very first j-steps) park on the DMA in the PE wait
                # queue and the rope raced ahead of the accumulation on HW
                xt_first[c + 1] = emit_xt_load(c + 1, 0)

        return ([mk_q(dt) for dt in range(DT)] + [mk_kv(dt) for dt in range(DT)]
                + [mk_tp])

    # ---- deferred normalization quanta for chunk c: Zinv broadcast (K=1
    # f32r matmul) + cast + multiply per (pair, half); ln/exp are emitted
    # immediately by the caller (ACT-only, off the PE queue) ----
    def norm_quanta(zi, pvs_list, attnT):
        def mk(p, half):
            def f():
                pvs = pvs_list[p]
                bc = psF.tile([HD, 512], F32, tag="fr")
                nc.tensor.matmul(bc[:], ones_r[:],
                                 zi[0:1, p * 1024 + half * 512:
                                    p * 1024 + (half + 1) * 512],
                                 skip_group_check=True)
                bcs = work.tile([HD, 512], BF16, tag="bc")
                nc.vector.tensor_copy(bcs[:], bc[:])
                if half == 0:
                    nc.vector.tensor_mul(attnT[0:HD, p * 512:(p + 1) * 512],
                                         pvs[0:HD, 0:512], bcs[:])
                else:
                    tmpb = work.tile([HD, 512], BF16, tag="tmpb", bufs=4)
                    nc.vector.tensor_mul(tmpb[:], pvs[0:HD, 512:1024], bcs[:])
                    # partition shift 0:64 -> 64:128 via sbuf-sbuf DMA
                    nc.sync.dma_start(attnT[HD:P, p * 512:(p + 1) * 512], tmpb[:])
            return f
        return [mk(p, h) for p in range(NPAIR) for h in range(2)]

    def emit_norm_prep(zg):
        """Batched softmax denominators: one Ln + one Exp for all 4 pairs.
        ln+exp live in one ACT table set with the scores' exp -> no reloads.
        The [4,1024] Zinv is DMA-flattened onto partition 0 because matmul
        rhs base partitions must be one of {0, 32, 64}."""
        zl = work.tile([4, 1024], F32, tag="zl", bufs=1)
        nc.scalar.activation(zl[:], zg[:], AFT.Ln)
        zi4 = work.tile([4, 1024], F32R, tag="zi4", bufs=1)
        nc.scalar.activation(zi4[:], zl[:], AFT.Exp, scale=-1.0)
        zi = work.tile([1, 4096], F32R, tag="zi", bufs=1)
        for p in range(NPAIR):
            nc.sync.dma_start(zi[0:1, p * 1024:(p + 1) * 1024], zi4[p:p + 1, :])
        return zi

    # ---- final matmul quanta: one PE matmul each; evacuate + output DMA
    # ride with the last accumulating matmul ----
    def final_quanta(c, attnT, st, tail=False):
        hold = {}

        def mk(dc, p):
            def f():
                if p == 0:
                    hold[dc] = psF.tile([P, 512], F32, tag="fr",
                                        name=f"rp_{c}_{st}_{dc}")
                rp = hold[dc]
                nc.tensor.matmul(rp[:], attnT[:, p * 512 + st * P: p * 512 + (st + 1) * P],
                                 woT[:, p * D + dc * 512: p * D + (dc + 1) * 512],
                                 start=(p == 0), stop=(p == NPAIR - 1),
                                 skip_group_check=True)
                if p == NPAIR - 1:
                    rs = work.tile([P, 512], BF16, tag="rs")
                    if tail and dc % 2 == 0:
                        # drain phase: ACT is idle, split evacuations
                        nc.scalar.copy(rs[:], rp[:])
                    else:
                        nc.vector.tensor_copy(rs[:], rp[:])
                    nc.sync.dma_start(out_d[(c * 4 + st) * P:(c * 4 + st + 1) * P,
                                            dc * 512:(dc + 1) * 512], rs[:])
            return f
        return [mk(dc, p) for dc in range(4) for p in range(NPAIR)]

    # ---- attention for one head pair; `pull` injects filler quanta between
    # the exp and the PV matmuls so the PE never idles on ACT latency ----
    def emit_attn_pair(c, p, qT, pull):
        NJT = 4 * (c + 1)
        pv = psV.tile([65, 1024], F32, tag="pv", name=f"pv_{c}_{p}")
        # j runs ascending: the first tiles only touch kT/qT of OLDER chunks,
        # giving the freshly-interleaved proj(c) ropes + xbar transposes the
        # whole pair runway before the diagonal tiles need them. (Since the
        # normalize is deferred a chunk, the gpsimd affine hop on the last
        # (diagonal) tiles only delays the evac, which nothing waits on.)
        for idx, j in enumerate(range(NJT - 1, -1, -1)):
            # causal: only columns q >= j*128 - c*512 within the chunk are live
            vs = max(0, (j - 4 * c) * P)
            w = 512 - vs
            sc2 = psS.tile([P, 1024], F32, tag="sc")
            nc.tensor.matmul(sc2[:, vs:512], kT[0:HD, j * P:(j + 1) * P],
                             qT[0:HD, p * 512 + vs:(p + 1) * 512])
            nc.tensor.matmul(sc2[:, 512 + vs:1024], kT[HD:P, j * P:(j + 1) * P],
                             qT[HD:P, p * 512 + vs:(p + 1) * 512])
            e2 = epool.tile([P, 1024], BF16, tag="e")
            e_v = e2[:].rearrange("p (h q) -> p h q", h=2, q=512)[:, :, vs:512]
            if vs:
                sc_v = sc2[:].rearrange("p (h q) -> p h q", h=2, q=512)[:, :, vs:512]
                nc.scalar.activation(e_v, sc_v, AFT.Exp, scale=1.0 / 8.0)
            else:
                nc.scalar.activation(e2[:], sc2[:], AFT.Exp, scale=1.0 / 8.0)
            if j >= 4 * c:  # diagonal block: zero where k_glob > q_glob
                # one merged op over both head-halves: iota resets per half
                nc.gpsimd.affine_select(
                    out=e_v, in_=e_v,
                    compare_op=mybir.AluOpType.is_ge, fill=0.0,
                    base=c * 512 + vs - j * P, channel_multiplier=-1,
                    pattern=[[0, 2], [1, w]])
            pull()
            nc.tensor.matmul(pv[:, vs:512], v2[:, j * 130: j * 130 + 65],
                             e2[:, vs:512],
                             start=(idx == 0), stop=(idx == NJT - 1), skip_group_check=True)
            nc.tensor.matmul(pv[:, 512 + vs:1024],
                             v2[:, j * 130 + 65: (j + 1) * 130],
                             e2[:, 512 + vs:1024],
                             start=(idx == 0), stop=(idx == NJT - 1), skip_group_check=True)
        # evacuate the accumulator to SBUF immediately so the single PV PSUM
        # slot frees for the next pair's j-loop. bf16 so downstream multiplies
        # hit the DVE 2x perf mode.
        pvs = pvp.tile([65, 1024], BF16, tag="pvs", name=f"pvs_{c}_{p}")
        nc.vector.tensor_copy(pvs[:], pv[:])
        return pvs

    # ---- main loop: chunk c's attention is interleaved (at single-matmul
    # granularity) with proj(c+1), final(c-1) and normalize(c-1) quanta ----
    qT_cur = qTp.tile([P, NPAIR * 512], BF16, tag="qT", name="qT_0")
    xts0_map = dict(enumerate(xts0))
    for st in range(4):
        for q in proj_quanta(0, qT_cur, st, xts0_map):
            q()

    pvs_prev = None     # chunk c-1 pair accumulators (unnormalized, bf16)
    zg_prev = None      # chunk c-1 Z rows [4, 1024]
    for c in range(NSC):
        qT = qT_cur
        last = c == NSC - 1
        items = []
        attnT = None
        if c >= 1:
            zi = emit_norm_prep(zg_prev)
            attnT = atp.tile([P, NPAIR * 512], BF16, tag="attnT", name=f"attnT_{c-1}")
            items += norm_quanta(zi, pvs_prev, attnT)
        if c + 1 < NSC:
            qT_cur = qTp.tile([P, NPAIR * 512], BF16, tag="qT", name=f"qT_{c+1}")
            xts = {0: xt_first.pop(c + 1)}
            pq = [proj_quanta(c + 1, qT_cur, st, xts) for st in range(4)]
        for st in range(4):
            if c + 1 < NSC:
                items += pq[st]
            if c >= 1:
                items += final_quanta(c - 1, attnT, st)
        filler = deque(items)
        steps = [4 * (c + 1) * NPAIR]  # j-steps left in this chunk

        def pull():
            n = (len(filler) + steps[0] - 1) // steps[0]
            if steps[0] > 1:
                steps[0] -= 1
            for _ in range(n):
                if filler:
                    filler.popleft()()

        pvs_list = []
        zg = zgp.tile([4, 1024], BF16, tag="zg", name=f"zg_{c}")
        for p in range(NPAIR):
            pvs = emit_attn_pair(c, p, qT, pull)
            pvs_list.append(pvs)
            nc.sync.dma_start(zg[p:p + 1, :], pvs[64:65, :])
        while filler:
            filler.popleft()()
        pvs_prev, zg_prev = pvs_list, zg

    # tail: normalize + final for the last chunk. NOTE: the final matmuls must
    # trail the attnT partition-shift DMAs by a few us (running them
    # back-to-back raced on hardware), which the 8 norm quanta provide.
    zi = emit_norm_prep(zg_prev)
    attnT = atp.tile([P, NPAIR * 512], BF16, tag="attnT", name=f"attnT_{NSC-1}")
    for q in norm_quanta(zi, pvs_prev, attnT):
        q()
    for st in range(4):
        for q in final_quanta(NSC - 1, attnT, st, tail=True):
            q()

    if DEBUG_DUMP:
        d_at = nc.dram_tensor("dbg_attnT", [P, NPAIR * 512], BF16,
                              kind="ExternalOutput").ap()
        nc.sync.dma_start(d_at, attnT[:])
        d_pv = nc.dram_tensor("dbg_pvs", [65, 4 * 1024], BF16,
                              kind="ExternalOutput").ap()
        for p in range(NPAIR):
            nc.sync.dma_start(d_pv[:, p * 1024:(p + 1) * 1024], pvs_prev[p][:])
        d_zi = nc.dram_tensor("dbg_zi", [1, 4096], F32, kind="ExternalOutput").ap()
        zf32 = work.tile([1, 4096], F32, tag="zf32", bufs=1)
        nc.vector.tensor_copy(zf32[:], zi[:])
        nc.sync.dma_start(d_zi, zf32[:])
        d_qt = nc.dram_tensor("dbg_qT", [P, NPAIR * 512], BF16,
                              kind="ExternalOutput").ap()
        nc.sync.dma_start(d_qt, qT_cur[:])
        d_kt = nc.dram_tensor("dbg_kT", [P, S], BF16, kind="ExternalOutput").ap()
        nc.sync.dma_start(d_kt, kT[:])


_NC_CACHE = {}


def _pin_exp_ln_table_set():
    """Make the ACT-table-load pass resolve both Exp and Ln to the one set
    that contains them both (natural_log_exp_and_others). The default
    first-containing-set choice alternates exp_and_others / natural_log per
    activation, inserting a ~1.3us table reload before every softmax
    normalization. Only the advertised membership used for set *selection*
    is filtered; set indices stay canonical, so the runtime tables match."""
    if getattr(bacc, "_exp_ln_pinned", False):
        return
    real = bacc.get_activation_tables

    def pinned(arch):
        tables = dict(real(arch))
        both = {AFT.Exp, AFT.Ln}
        for name in list(tables):
            if name != "natural_log_exp_and_others" and (tables[name] & both):
                tables[name] = tables[name] - both
        return tables

    bacc.get_activation_tables = pinned
    bacc._exp_ln_pinned = True


def build(S=2048):
    if S in _NC_CACHE:
        return _NC_CACHE[S]
    from contextlib import ExitStack
    _pin_exp_ln_table_set()
    nc = bacc.Bacc("TRN2", target_bir_lowering=False, debug=False, num_devices=8)
    with tile.TileContext(nc) as tc, ExitStack() as ctx:
        emit_kernel(nc, tc, ctx, S)
    nc.compile()
    _NC_CACHE[S] = nc
    return nc


def shard_inputs(x, theta, wq, wk, wv, wo, S=2048):
    """Returns in_maps for 8 cores: core = b*4 + g. Pure layout prep."""
    cost = np.cos(theta[:S]).astype(np.float32)
    sint = np.sin(theta[:S]).astype(np.float32)
    in_maps = []
    for core in range(8):
        b, g = core // 4, core % 4
        wq_g = wq[g * 512:(g + 1) * 512].reshape(8, HD, D)[HEAD_PERM].reshape(512, D)
        wo_g = wo[:, g * 512:(g + 1) * 512].reshape(D, 8, HD)[:, HEAD_PERM].reshape(D, 512)
        wkv_g = np.concatenate([wk[g * 128:(g + 1) * 128], wv[g * 128:(g + 1) * 128]], axis=0)
        bf = ml_dtypes.bfloat16
        in_maps.append({
            "xT": np.ascontiguousarray(x[b, :S].T).astype(bf),
            "wqT": np.ascontiguousarray(wq_g.T).astype(bf),
            "wkvT": np.ascontiguousarray(wkv_g.T).astype(bf),
            "woT": np.ascontiguousarray(wo_g.T).astype(bf),
            "cost": cost,
            "sint": sint,
        })
    return in_maps


def run_on_hw(inputs, S=2048, trace=False):
    nc = build(S)
    in_maps = shard_inputs(inputs["x"], inputs["theta"], inputs["wq"],
                           inputs["wk"], inputs["wv"], inputs["wo"], S=S)
    res = bass_utils.run_bass_kernel_spmd(nc, in_maps, core_ids=list(range(8)),
                                          trace=trace)
    parts = [res.results[c]["out"].astype(np.float32) for c in range(8)]
    out = np.stack([parts[0] + parts[1] + parts[2] + parts[3],
                    parts[4] + parts[5] + parts[6] + parts[7]], axis=0)
    return out, res


def kernel(x, theta, mask, wq, wk, wv, wo):
    out, _ = run_on_hw({"x": np.asarray(x, np.float32), "theta": np.asarray(theta, np.float32),
                        "wq": np.asarray(wq, np.float32), "wk": np.asarray(wk, np.float32),
                        "wv": np.asarray(wv, np.float32), "wo": np.asarray(wo, np.float32)})
    return out
